# revision 8
# baseline (speedup 1.0000x reference)
"""GAT attention kernel for 8 trn2 NeuronCores (Bass/Tile), bf16 data path.

Math (restructured from the reference to avoid materializing h_j):
    wa1 = W @ a1, wa2 = W @ a2                      (host, fp32)
    s[n,k]  = x0[n]·wa1 + x[n,k]·wa2                (since h@a1 = x0@(W a1))
    e       = leaky_relu(s, 0.2)
    p       = exp(e) * adj                          (no max-sub: scores are small)
    att     = (p + EPS) * recip(sum_k p + 16*EPS)   (== uniform 1/16 when row fully
                                                     masked, matching reference)
    xbar[n] = sum_k att[n,k] * x[n,k,:]
    out     = elu((xbar + x0) @ W)                  (since h_prime + h = (xbar + x0)@W)
    elu(z)  = min(exp(z) - 1, relu(z))

Sharding: node dim N padded 50000 -> 50176 = 8 cores * 49 tiles * 128 rows.
Per 128-row tile the 2048 (n,k) pairs form 16 blocks of [128 nk-rows, 128 feat]
held as x_tile[:, b*128:(b+1)*128] in bf16 (host pre-permutes + converts so the
DMA is a single contiguous ~560KB transfer per tile). Block b, partition
q = (m%8)*16 + k holds x[node 8b + q//16, k = q%16, :].

All matmul operands are bf16 (single PE pass vs 2 for fp32); PSUM stays fp32.
Scores for all 16 blocks + the self term are ONE DVE multiply [128, 2176]
against a host-packed [wa2 x16 | wa1] row followed by ONE DVE tensor_reduce
(innermost-axis segment sum), replacing 17 per-block STT reductions.

Per tile:
  DVE : score mult + segment-reduce, reciprocal, att STT, attseg broadcast TT
  PE  : si scatter (Cm), Z group-sum (SEG), RZ broadcast (E8),
        x0^T identity + 16 xbar block matmuls (PSUM accumulate), final
        (xbar+x0)^T.T @ W
  ACT : si/tz PSUM copies, exp(ls), ST copy (bf16), exp/relu of final
  POOL: Dt scatter mul, s2 add, leaky-relu STT, p=exp*adj, elu-tail STT
"""

import numpy as np

N, K, F = 50000, 16, 128
ALPHA = 0.2
NCORES = 8
TILE = 128
NTILES = 49
RPC = TILE * NTILES          # rows per core = 6272
BPT = K                      # nk-blocks per tile = 16
XCOLS = BPT * F + F + K      # x blocks + x0 + adj = 2192 cols (bf16)
WACOLS = (BPT + 1) * F       # wa2 x16 | wa1 = 2176
CCOLS = 128 * 4 + 8 + 16 + 128 + WACOLS  # ident,Cm,segbig,e8 + seg + seg8 + W + wa
EPS = 1e-12

_NC_CACHE = {}


def _bf16():
    import ml_dtypes
    return ml_dtypes.bfloat16


def _consts_np(W, a):
    p = np.arange(128)
    j8 = np.arange(8)
    b16 = np.arange(16)
    ident = np.eye(128, dtype=np.float32)
    # C[n, q] = 1 iff n%8 == q//16   (si scatter: out[q,b] = si[8b + q//16])
    Cm = (p[:, None] % 8 == p[None, :] // 16).astype(np.float32)
    # SEGBIG[q, 8b+j] = 1 iff j == q//16  (pattern repeats over b)
    segbig = (p[:, None] // 16 == (p[None, :] % 8)).astype(np.float32)
    # E8[j, q] = 1 iff q//16 == j (rows 8..127 zero; used as lhsT [8,128])
    e8 = ((p[:, None] < 8) & (p[None, :] // 16 == p[:, None])).astype(np.float32)
    # SEG[q, j] = 1 iff q//16 == j   [128, 8]
    seg = (p[:, None] // 16 == j8[None, :]).astype(np.float32)
    # SEG8[n, b] = 1 iff n//8 == b   [128, 16]
    seg8 = (p[:, None] // 8 == b16[None, :]).astype(np.float32)
    W = np.asarray(W, np.float32)
    a = np.asarray(a, np.float32)
    wa1 = W @ a[:F, 0]           # [F]
    wa2 = W @ a[F:, 0]           # [F]
    warow = np.concatenate([np.tile(wa2, BPT), wa1])  # [2176]
    wab = np.broadcast_to(warow, (128, WACOLS))
    return np.ascontiguousarray(
        np.concatenate([ident, Cm, segbig, e8, seg, seg8, W, wab], axis=1)
    ).astype(_bf16())  # [128, CCOLS]


def _build_nc(ntiles=NTILES):
    import concourse.mybir as mybir
    import concourse.tile as tile
    from concourse import bacc

    fp = mybir.dt.float32
    bf = mybir.dt.bfloat16
    AF = mybir.ActivationFunctionType
    OP = mybir.AluOpType
    AX = mybir.AxisListType

    nc = bacc.Bacc("TRN2")
    xd = nc.dram_tensor("xd", [ntiles, 128, XCOLS], bf, kind="ExternalInput")
    cst = nc.dram_tensor("cst", [128, CCOLS], bf, kind="ExternalInput")
    yd = nc.dram_tensor("yd", [ntiles, 128, F], fp, kind="ExternalOutput")

    with tile.TileContext(nc) as tc:
        with (
            tc.tile_pool(name="const", bufs=1) as constp,
            tc.tile_pool(name="xin", bufs=7) as xin,
            tc.tile_pool(name="prodp", bufs=3) as prodp,
            tc.tile_pool(name="small", bufs=4) as small,
            tc.tile_pool(name="big", bufs=3) as big,
            tc.tile_pool(name="yout", bufs=3) as yout,
            # one PSUM pool; per-tag bufs: si 1 + Z 2 + RZrep 2 + mm 3 = 8 banks
            tc.tile_pool(name="ps", bufs=1, space="PSUM") as ps,
        ):
            # ---------------- setup: single DMA, no device-side prep ------
            consts = constp.tile([128, CCOLS], bf)
            nc.sync.dma_start(out=consts, in_=cst[:, :])
            o = 0
            IDENT = consts[:, o:o + 128]; o += 128
            Cm = consts[:, o:o + 128]; o += 128
            SEGBIG = consts[:, o:o + 128]; o += 128
            E8 = consts[:, o:o + 128]; o += 128
            SEG = consts[:, o:o + 8]; o += 8
            SEG8 = consts[:, o:o + 16]; o += 16
            W_sb = consts[:, o:o + 128]; o += 128
            WAB = consts[:, o:o + WACOLS]; o += WACOLS

            # ---------------- software-pipelined tile loop ----------------
            #   load(t) | scores(t-2) | mask(t-3) | recip(t-4) | out(t-5)
            st = {}

            def phase_load(t):
                xall = xin.tile([128, XCOLS], bf, tag="x")
                nc.sync.dma_start(out=xall, in_=xd[t])
                st[t] = {"xall": xall}

            def phase_score(t):
                d = st[t]
                xall = d["xall"]
                # one multiply + one segment reduce covers all 16 block
                # scores AND the x0·wa1 self term (block 16).
                prod = prodp.tile([128, WACOLS], bf, tag="prod")
                # split the big multiply across DVE and Pool (disjoint cols)
                H = WACOLS // 2
                nc.vector.tensor_mul(out=prod[:, 0:H], in0=xall[:, 0:H],
                                     in1=WAB[:, 0:H])
                nc.gpsimd.tensor_mul(out=prod[:, H:WACOLS], in0=xall[:, H:WACOLS],
                                     in1=WAB[:, H:WACOLS])
                # DVE reduce runs at f32 rate (no 2x for bf16) while TT adds
                # get 2x: fold 128 -> 32 with two bf16 tree adds, then reduce
                # the narrow remainder.
                p3 = prod.rearrange("p (b f) -> p b f", f=F)
                u1 = prodp.tile([128, 17 * 64], bf, tag="u1")
                u13 = u1.rearrange("p (b f) -> p b f", f=64)
                nc.vector.tensor_add(out=u13, in0=p3[:, :, 0:64], in1=p3[:, :, 64:128])
                u2 = prodp.tile([128, 17 * 32], bf, tag="u2")
                u23 = u2.rearrange("p (b f) -> p b f", f=32)
                nc.vector.tensor_add(out=u23, in0=u13[:, :, 0:32], in1=u13[:, :, 32:64])
                sboth = small.tile([128, BPT + 1], fp, tag="sboth")
                nc.vector.tensor_reduce(
                    out=sboth, in_=u23, axis=AX.X, op=OP.add,
                )
                si_nat = sboth[:, BPT:BPT + 1]
                Dt = small.tile([128, K], bf, tag="D")
                # Pool rejects bf16 TensorScalarPtr; DVE takes bf16 data +
                # f32 scalar AP.
                nc.vector.tensor_scalar_mul(out=Dt, in0=SEG8, scalar1=si_nat)
                si_ps = ps.tile([128, K], fp, tag="si", bufs=1)
                nc.tensor.matmul(si_ps, lhsT=Cm, rhs=Dt, start=True, stop=True)
                si_s = small.tile([128, K], fp, tag="si_s")
                nc.scalar.activation(out=si_s, in_=si_ps, func=AF.Copy)
                s2 = small.tile([128, K], fp, tag="s2")
                nc.gpsimd.tensor_add(out=s2, in0=sboth[:, 0:BPT], in1=si_s)
                d["s2"] = s2

            def phase_mask(t):
                d = st[t]
                adj_f = d["xall"][:, BPT * F + F:XCOLS]
                # Pool has no scalar_tensor_tensor on TRN2 -> DVE
                ls = small.tile([128, K], fp, tag="ls")
                nc.vector.scalar_tensor_tensor(
                    out=ls, in0=d["s2"], scalar=ALPHA, in1=d["s2"],
                    op0=OP.mult, op1=OP.max,
                )
                exp_s = small.tile([128, K], bf, tag="exp_s")
                nc.scalar.activation(out=exp_s, in_=ls, func=AF.Exp)
                p_s = small.tile([128, K], bf, tag="p_s")
                nc.gpsimd.tensor_mul(out=p_s, in0=exp_s, in1=adj_f)
                Z_ps = ps.tile([8, K], fp, tag="Z", bufs=2)
                nc.tensor.matmul(Z_ps, lhsT=SEG, rhs=p_s, start=True, stop=True)
                tz = small.tile([8, K], fp, tag="tz")
                nc.scalar.activation(out=tz, in_=Z_ps, func=AF.Copy, bias=16.0 * EPS)
                d["p_s"] = p_s
                d["tz"] = tz

            def phase_recip(t):
                d = st[t]
                RZ = small.tile([8, K], bf, tag="RZ")
                with nc.allow_low_precision("bf16 attention normalizer"):
                    nc.vector.reciprocal(RZ, d["tz"])
                RZrep_ps = ps.tile([128, K], fp, tag="RZrep", bufs=2)
                nc.tensor.matmul(RZrep_ps, lhsT=E8[0:8, :], rhs=RZ,
                                 start=True, stop=True)
                d["RZrep"] = RZrep_ps

            def phase_out(t):
                d = st[t]
                xall = d["xall"]
                x0_tile = xall[:, BPT * F:BPT * F + F]
                att = small.tile([128, K], bf, tag="att")
                nc.vector.scalar_tensor_tensor(
                    out=att, in0=d["p_s"], scalar=EPS, in1=d["RZrep"],
                    op0=OP.add, op1=OP.mult,
                )
                attseg = big.tile([128, 128], bf, tag="attseg")
                att_bc = att.rearrange("p (b o) -> p b o", o=1).to_broadcast([128, K, 8])
                nc.gpsimd.tensor_mul(
                    out=attseg.rearrange("p (b j) -> p b j", j=8),
                    in0=SEGBIG.rearrange("p (b j) -> p b j", j=8),
                    in1=att_bc,
                )
                xbarT_ps = ps.tile([128, 128], fp, tag="mm", bufs=3)
                nc.tensor.matmul(xbarT_ps, lhsT=x0_tile, rhs=IDENT,
                                 start=True, stop=False, skip_group_check=True)
                for b in range(BPT):
                    nc.tensor.matmul(
                        xbarT_ps[:, 8 * b:8 * b + 8],
                        lhsT=xall[:, b * F:(b + 1) * F],
                        rhs=attseg[:, 8 * b:8 * b + 8],
                        start=False, stop=(b == BPT - 1), skip_group_check=True,
                    )
                ST_sb = big.tile([128, 128], bf, tag="ST")
                nc.scalar.activation(out=ST_sb, in_=xbarT_ps, func=AF.Copy)
                zfin_ps = ps.tile([128, 128], fp, tag="mm", bufs=3)
                nc.tensor.matmul(zfin_ps, lhsT=ST_sb, rhs=W_sb, start=True, stop=True)
                e_sb = big.tile([128, 128], fp, tag="e")
                nc.scalar.activation(out=e_sb, in_=zfin_ps, func=AF.Exp)
                r_sb = big.tile([128, 128], fp, tag="r")
                nc.scalar.activation(out=r_sb, in_=zfin_ps, func=AF.Relu)
                y_sb = yout.tile([128, 128], fp, tag="y")
                nc.vector.scalar_tensor_tensor(
                    out=y_sb, in0=e_sb, scalar=1.0, in1=r_sb,
                    op0=OP.subtract, op1=OP.min,
                )
                nc.sync.dma_start(out=yd[t], in_=y_sb)
                del st[t]

            for r in range(ntiles + 5):
                if r < ntiles:
                    phase_load(r)
                if 0 <= r - 2 < ntiles:
                    phase_score(r - 2)
                if 0 <= r - 3 < ntiles:
                    phase_mask(r - 3)
                if 0 <= r - 4 < ntiles:
                    phase_recip(r - 4)
                if 0 <= r - 5 < ntiles:
                    phase_out(r - 5)

    nc.finalize()
    return nc


def _get_nc(ntiles=NTILES):
    if ntiles not in _NC_CACHE:
        _NC_CACHE[ntiles] = _build_nc(ntiles)
    return _NC_CACHE[ntiles]


def _shard_inputs(orignal_x, x, adj, W, a, ncores=NCORES, ntiles=NTILES):
    bf16 = _bf16()
    rpc = TILE * ntiles
    n_used = rpc * ncores
    x = np.asarray(x, np.float32).astype(bf16)
    x0 = np.asarray(orignal_x, np.float32).astype(bf16)
    adjb = np.asarray(adj, np.int32).astype(bf16)
    consts = _consts_np(W, a)
    n = x.shape[0]

    in_maps = []
    for c in range(ncores):
        lo = c * rpc
        hi = min((c + 1) * rpc, n)
        rows = hi - lo
        xc = x[lo:hi]
        x0c = x0[lo:hi]
        adjc = adjb[lo:hi]
        if rows < rpc:
            pad = rpc - rows
            xc = np.concatenate([xc, np.zeros((pad, K, F), bf16)])
            x0c = np.concatenate([x0c, np.zeros((pad, F), bf16)])
            adjc = np.concatenate([adjc, np.zeros((pad, K), bf16)])
        # per-tile layout [t, p, b*F+f] with x0 and adj packed as trailing
        # columns (adj in s-layout: adj_s[q, b] = adj_flat[128b+q])
        xdev = np.empty((ntiles, 128, XCOLS), bf16)
        xdev[:, :, :BPT * F] = xc.reshape(ntiles, BPT, 128, F).transpose(
            0, 2, 1, 3).reshape(ntiles, 128, BPT * F)
        xdev[:, :, BPT * F:BPT * F + F] = x0c.reshape(ntiles, 128, F)
        xdev[:, :, BPT * F + F:] = adjc.reshape(ntiles, BPT, 128).transpose(0, 2, 1)
        in_maps.append({
            "xd": xdev,
            "cst": consts,
        })
    assert n <= n_used
    return in_maps


_LAST_RESULTS = None


def kernel(orignal_x, x, adj, W, a):
    import os
    os.environ.setdefault("JAX_PLATFORMS", "")
    from concourse.bass_utils import run_bass_kernel_spmd

    global _LAST_RESULTS
    nc = _get_nc()
    in_maps = _shard_inputs(orignal_x, x, adj, W, a)
    res = run_bass_kernel_spmd(nc, in_maps, list(range(NCORES)))
    _LAST_RESULTS = res
    y = np.concatenate([r["yd"].reshape(RPC, F) for r in res.results], axis=0)
    return np.ascontiguousarray(y[:N])


# revision 9
# speedup vs baseline: 1.0827x; 1.0827x over previous
"""GAT attention kernel for 8 trn2 NeuronCores (Bass/Tile), bf16 data path.

Math (restructured from the reference to avoid materializing h_j):
    wa1 = W @ a1, wa2 = W @ a2                      (host, fp32)
    s[n,k]  = x0[n]·wa1 + x[n,k]·wa2                (since h@a1 = x0@(W a1))
    e       = leaky_relu(s, 0.2)
    p       = exp(e) * adj                          (no max-sub: scores are small)
    att     = (p + EPS) * recip(sum_k p + 16*EPS)   (== uniform 1/16 when row fully
                                                     masked, matching reference)
    xbar[n] = sum_k att[n,k] * x[n,k,:]
    out     = elu((xbar + x0) @ W)                  (since h_prime + h = (xbar + x0)@W)
    elu(z)  = min(exp(z) - 1, relu(z))

Sharding: node dim N padded 50000 -> 50176 = 8 cores * 49 tiles * 128 rows.
Per 128-row tile the 2048 (n,k) pairs form 16 blocks of [128 nk-rows, 128 feat]
held as x_tile[:, b*128:(b+1)*128] in bf16 (host pre-permutes + converts so the
DMA is a single contiguous ~560KB transfer per tile). Block b, partition
q = (m%8)*16 + k holds x[node 8b + q//16, k = q%16, :].

All matmul operands are bf16 (single PE pass vs 2 for fp32); PSUM stays fp32.
Scores for all 16 blocks + the self term are ONE DVE multiply [128, 2176]
against a host-packed [wa2 x16 | wa1] row followed by ONE DVE tensor_reduce
(innermost-axis segment sum), replacing 17 per-block STT reductions.

Per tile:
  DVE : score mult + segment-reduce, reciprocal, att STT, attseg broadcast TT
  PE  : si scatter (Cm), Z group-sum (SEG), RZ broadcast (E8),
        x0^T identity + 16 xbar block matmuls (PSUM accumulate), final
        (xbar+x0)^T.T @ W
  ACT : si/tz PSUM copies, exp(ls), ST copy (bf16), exp/relu of final
  POOL: Dt scatter mul, s2 add, leaky-relu STT, p=exp*adj, elu-tail STT
"""

import numpy as np

N, K, F = 50000, 16, 128
ALPHA = 0.2
NCORES = 8
TILE = 128
NTILES = 49
RPC = TILE * NTILES          # rows per core = 6272
BPT = K                      # nk-blocks per tile = 16
XCOLS = BPT * F + F + K      # x blocks + x0 + adj = 2192 cols (bf16)
WACOLS = (BPT + 1) * F       # wa2 x16 | wa1 = 2176
CCOLS = 128 * 4 + 8 + 16 + 128 + WACOLS  # ident,Cm,segbig,e8 + seg + seg8 + W + wa
EPS = 1e-12

_NC_CACHE = {}


def _bf16():
    import ml_dtypes
    return ml_dtypes.bfloat16


def _consts_np(W, a):
    p = np.arange(128)
    j8 = np.arange(8)
    b16 = np.arange(16)
    ident = np.eye(128, dtype=np.float32)
    # C[n, q] = 1 iff n%8 == q//16   (si scatter: out[q,b] = si[8b + q//16])
    Cm = (p[:, None] % 8 == p[None, :] // 16).astype(np.float32)
    # SEGBIG[q, 8b+j] = 1 iff j == q//16  (pattern repeats over b)
    segbig = (p[:, None] // 16 == (p[None, :] % 8)).astype(np.float32)
    # E8[j, q] = 1 iff q//16 == j (rows 8..127 zero; used as lhsT [8,128])
    e8 = ((p[:, None] < 8) & (p[None, :] // 16 == p[:, None])).astype(np.float32)
    # SEG[q, j] = 1 iff q//16 == j   [128, 8]
    seg = (p[:, None] // 16 == j8[None, :]).astype(np.float32)
    # SEG8[n, b] = 1 iff n//8 == b   [128, 16]
    seg8 = (p[:, None] // 8 == b16[None, :]).astype(np.float32)
    W = np.asarray(W, np.float32)
    a = np.asarray(a, np.float32)
    wa1 = W @ a[:F, 0]           # [F]
    wa2 = W @ a[F:, 0]           # [F]
    warow = np.concatenate([np.tile(wa2, BPT), wa1])  # [2176]
    wab = np.broadcast_to(warow, (128, WACOLS))
    return np.ascontiguousarray(
        np.concatenate([ident, Cm, segbig, e8, seg, seg8, W, wab], axis=1)
    ).astype(_bf16())  # [128, CCOLS]


def _build_nc(ntiles=NTILES):
    import concourse.mybir as mybir
    import concourse.tile as tile
    from concourse import bacc

    fp = mybir.dt.float32
    bf = mybir.dt.bfloat16
    AF = mybir.ActivationFunctionType
    OP = mybir.AluOpType
    AX = mybir.AxisListType

    nc = bacc.Bacc("TRN2")
    xd = nc.dram_tensor("xd", [ntiles, 128, XCOLS], bf, kind="ExternalInput")
    cst = nc.dram_tensor("cst", [128, CCOLS], bf, kind="ExternalInput")
    yd = nc.dram_tensor("yd", [ntiles, 128, F], fp, kind="ExternalOutput")

    with tile.TileContext(nc) as tc:
        with (
            tc.tile_pool(name="const", bufs=1) as constp,
            tc.tile_pool(name="xin", bufs=7) as xin,
            tc.tile_pool(name="prodp", bufs=3) as prodp,
            tc.tile_pool(name="small", bufs=4) as small,
            tc.tile_pool(name="big", bufs=3) as big,
            tc.tile_pool(name="yout", bufs=3) as yout,
            # one PSUM pool; per-tag bufs: si 1 + Z 2 + RZrep 2 + mm 3 = 8 banks
            tc.tile_pool(name="ps", bufs=1, space="PSUM") as ps,
        ):
            # ---------------- setup: single DMA, no device-side prep ------
            consts = constp.tile([128, CCOLS], bf)
            nc.sync.dma_start(out=consts, in_=cst[:, :])
            o = 0
            IDENT = consts[:, o:o + 128]; o += 128
            Cm = consts[:, o:o + 128]; o += 128
            SEGBIG = consts[:, o:o + 128]; o += 128
            E8 = consts[:, o:o + 128]; o += 128
            SEG = consts[:, o:o + 8]; o += 8
            SEG8 = consts[:, o:o + 16]; o += 16
            W_sb = consts[:, o:o + 128]; o += 128
            WAB = consts[:, o:o + WACOLS]; o += WACOLS

            # ---------------- software-pipelined tile loop ----------------
            #   load(t) | scores(t-2) | mask(t-3) | recip(t-4) | out(t-5)
            st = {}

            def phase_load(t):
                xall = xin.tile([128, XCOLS], bf, tag="x")
                nc.sync.dma_start(out=xall, in_=xd[t])
                st[t] = {"xall": xall}

            def phase_score(t):
                d = st[t]
                xall = d["xall"]
                # one multiply + one segment reduce covers all 16 block
                # scores AND the x0·wa1 self term (block 16).
                prod = prodp.tile([128, WACOLS], bf, tag="prod")
                nc.vector.tensor_mul(out=prod, in0=xall[:, 0:WACOLS], in1=WAB)
                # DVE reduce runs at f32 rate (no 2x for bf16) while TT adds
                # get 2x: fold 128 -> 32 with two bf16 tree adds, then reduce
                # the narrow remainder.
                p3 = prod.rearrange("p (b f) -> p b f", f=F)
                u1 = prodp.tile([128, 17 * 64], bf, tag="u1")
                u13 = u1.rearrange("p (b f) -> p b f", f=64)
                nc.vector.tensor_add(out=u13, in0=p3[:, :, 0:64], in1=p3[:, :, 64:128])
                u2 = prodp.tile([128, 17 * 32], bf, tag="u2")
                u23 = u2.rearrange("p (b f) -> p b f", f=32)
                nc.vector.tensor_add(out=u23, in0=u13[:, :, 0:32], in1=u13[:, :, 32:64])
                sboth = small.tile([128, BPT + 1], fp, tag="sboth")
                nc.vector.tensor_reduce(
                    out=sboth, in_=u23, axis=AX.X, op=OP.add,
                )
                si_nat = sboth[:, BPT:BPT + 1]
                Dt = small.tile([128, K], bf, tag="D")
                # Pool rejects bf16 TensorScalarPtr; DVE takes bf16 data +
                # f32 scalar AP.
                nc.vector.tensor_scalar_mul(out=Dt, in0=SEG8, scalar1=si_nat)
                si_ps = ps.tile([128, K], fp, tag="si", bufs=1)
                nc.tensor.matmul(si_ps, lhsT=Cm, rhs=Dt, start=True, stop=True)
                si_s = small.tile([128, K], fp, tag="si_s")
                nc.scalar.activation(out=si_s, in_=si_ps, func=AF.Copy)
                s2 = small.tile([128, K], fp, tag="s2")
                nc.gpsimd.tensor_add(out=s2, in0=sboth[:, 0:BPT], in1=si_s)
                d["s2"] = s2

            def phase_mask(t):
                d = st[t]
                adj_f = d["xall"][:, BPT * F + F:XCOLS]
                # Pool has no scalar_tensor_tensor on TRN2 -> DVE
                ls = small.tile([128, K], fp, tag="ls")
                nc.vector.scalar_tensor_tensor(
                    out=ls, in0=d["s2"], scalar=ALPHA, in1=d["s2"],
                    op0=OP.mult, op1=OP.max,
                )
                exp_s = small.tile([128, K], bf, tag="exp_s")
                nc.scalar.activation(out=exp_s, in_=ls, func=AF.Exp)
                p_s = small.tile([128, K], bf, tag="p_s")
                nc.gpsimd.tensor_mul(out=p_s, in0=exp_s, in1=adj_f)
                Z_ps = ps.tile([8, K], fp, tag="Z", bufs=2)
                nc.tensor.matmul(Z_ps, lhsT=SEG, rhs=p_s, start=True, stop=True)
                tz = small.tile([8, K], fp, tag="tz")
                nc.scalar.activation(out=tz, in_=Z_ps, func=AF.Copy, bias=16.0 * EPS)
                d["p_s"] = p_s
                d["tz"] = tz

            def phase_recip(t):
                d = st[t]
                RZ = small.tile([8, K], bf, tag="RZ")
                with nc.allow_low_precision("bf16 attention normalizer"):
                    nc.vector.reciprocal(RZ, d["tz"])
                RZrep_ps = ps.tile([128, K], fp, tag="RZrep", bufs=2)
                nc.tensor.matmul(RZrep_ps, lhsT=E8[0:8, :], rhs=RZ,
                                 start=True, stop=True)
                d["RZrep"] = RZrep_ps

            def phase_out(t):
                d = st[t]
                xall = d["xall"]
                x0_tile = xall[:, BPT * F:BPT * F + F]
                att = small.tile([128, K], bf, tag="att")
                nc.vector.scalar_tensor_tensor(
                    out=att, in0=d["p_s"], scalar=EPS, in1=d["RZrep"],
                    op0=OP.add, op1=OP.mult,
                )
                attseg = big.tile([128, 128], bf, tag="attseg")
                att_bc = att.rearrange("p (b o) -> p b o", o=1).to_broadcast([128, K, 8])
                nc.gpsimd.tensor_mul(
                    out=attseg.rearrange("p (b j) -> p b j", j=8),
                    in0=SEGBIG.rearrange("p (b j) -> p b j", j=8),
                    in1=att_bc,
                )
                xbarT_ps = ps.tile([128, 128], fp, tag="mm", bufs=3)
                nc.tensor.matmul(xbarT_ps, lhsT=x0_tile, rhs=IDENT,
                                 start=True, stop=False, skip_group_check=True)
                for b in range(BPT):
                    nc.tensor.matmul(
                        xbarT_ps[:, 8 * b:8 * b + 8],
                        lhsT=xall[:, b * F:(b + 1) * F],
                        rhs=attseg[:, 8 * b:8 * b + 8],
                        start=False, stop=(b == BPT - 1), skip_group_check=True,
                    )
                ST_sb = big.tile([128, 128], bf, tag="ST")
                nc.scalar.activation(out=ST_sb, in_=xbarT_ps, func=AF.Copy)
                zfin_ps = ps.tile([128, 128], fp, tag="mm", bufs=3)
                nc.tensor.matmul(zfin_ps, lhsT=ST_sb, rhs=W_sb, start=True, stop=True)
                e_sb = big.tile([128, 128], fp, tag="e")
                nc.scalar.activation(out=e_sb, in_=zfin_ps, func=AF.Exp)
                r_sb = big.tile([128, 128], fp, tag="r")
                nc.scalar.activation(out=r_sb, in_=zfin_ps, func=AF.Relu)
                y_sb = yout.tile([128, 128], fp, tag="y")
                nc.vector.scalar_tensor_tensor(
                    out=y_sb, in0=e_sb, scalar=1.0, in1=r_sb,
                    op0=OP.subtract, op1=OP.min,
                )
                nc.sync.dma_start(out=yd[t], in_=y_sb)
                del st[t]

            for r in range(ntiles + 5):
                if r < ntiles:
                    phase_load(r)
                if 0 <= r - 2 < ntiles:
                    phase_score(r - 2)
                if 0 <= r - 3 < ntiles:
                    phase_mask(r - 3)
                if 0 <= r - 4 < ntiles:
                    phase_recip(r - 4)
                if 0 <= r - 5 < ntiles:
                    phase_out(r - 5)

    nc.finalize()
    return nc


def _get_nc(ntiles=NTILES):
    if ntiles not in _NC_CACHE:
        _NC_CACHE[ntiles] = _build_nc(ntiles)
    return _NC_CACHE[ntiles]


def _shard_inputs(orignal_x, x, adj, W, a, ncores=NCORES, ntiles=NTILES):
    bf16 = _bf16()
    rpc = TILE * ntiles
    n_used = rpc * ncores
    x = np.asarray(x, np.float32).astype(bf16)
    x0 = np.asarray(orignal_x, np.float32).astype(bf16)
    adjb = np.asarray(adj, np.int32).astype(bf16)
    consts = _consts_np(W, a)
    n = x.shape[0]

    in_maps = []
    for c in range(ncores):
        lo = c * rpc
        hi = min((c + 1) * rpc, n)
        rows = hi - lo
        xc = x[lo:hi]
        x0c = x0[lo:hi]
        adjc = adjb[lo:hi]
        if rows < rpc:
            pad = rpc - rows
            xc = np.concatenate([xc, np.zeros((pad, K, F), bf16)])
            x0c = np.concatenate([x0c, np.zeros((pad, F), bf16)])
            adjc = np.concatenate([adjc, np.zeros((pad, K), bf16)])
        # per-tile layout [t, p, b*F+f] with x0 and adj packed as trailing
        # columns (adj in s-layout: adj_s[q, b] = adj_flat[128b+q])
        xdev = np.empty((ntiles, 128, XCOLS), bf16)
        xdev[:, :, :BPT * F] = xc.reshape(ntiles, BPT, 128, F).transpose(
            0, 2, 1, 3).reshape(ntiles, 128, BPT * F)
        xdev[:, :, BPT * F:BPT * F + F] = x0c.reshape(ntiles, 128, F)
        xdev[:, :, BPT * F + F:] = adjc.reshape(ntiles, BPT, 128).transpose(0, 2, 1)
        in_maps.append({
            "xd": xdev,
            "cst": consts,
        })
    assert n <= n_used
    return in_maps


_LAST_RESULTS = None


def kernel(orignal_x, x, adj, W, a):
    import os
    os.environ.setdefault("JAX_PLATFORMS", "")
    from concourse.bass_utils import run_bass_kernel_spmd

    global _LAST_RESULTS
    nc = _get_nc()
    in_maps = _shard_inputs(orignal_x, x, adj, W, a)
    res = run_bass_kernel_spmd(nc, in_maps, list(range(NCORES)))
    _LAST_RESULTS = res
    y = np.concatenate([r["yd"].reshape(RPC, F) for r in res.results], axis=0)
    return np.ascontiguousarray(y[:N])
